# revision 1
# baseline (speedup 1.0000x reference)
"""Trainium2 Bass kernel for nn_AttentionFusion.

reference:
    s1 = x1 @ w; s2 = x2 @ w               # [N]
    a1 = sigmoid(s1 - s2); a2 = 1 - a1     # [N]
    fused = a1[:,None]*x1 + a2[:,None]*x2  # [N, 64]
    alpha = stack([a1, a2], axis=1)        # [N, 2]

Sharding: rows split evenly across 8 NeuronCores (125000 rows/core), w
replicated. Per core the rows are streamed in a flat layout: each SBUF
partition holds R consecutive rows (R*64 contiguous fp32 -> big linear
DMA bursts). Compute per chunk:
    diff  = x1 - x2                       (DVE)
    sdiff = reduce_d(diff * w)            (DVE mul + grouped reduce)
    a1    = sigmoid(sdiff), a2 = sigmoid(-sdiff)   (ACT)
    fused = x2 + a1 * diff                (DVE, a1 broadcast along d)
"""

import numpy as np
from contextlib import ExitStack

import concourse.bass as bass
import concourse.bacc as bacc
import concourse.tile as tile
from concourse import mybir
from concourse.bass_utils import run_bass_kernel_spmd

N_CORES = 8
N, D = 1_000_000, 64
RN = N // N_CORES            # 125000 rows per core
P = 128                      # SBUF partitions
R = 32                       # rows per partition per main chunk
CHUNK_ROWS = P * R           # 4096
N_CHUNKS = RN // CHUNK_ROWS  # 30
MAIN_ROWS = N_CHUNKS * CHUNK_ROWS  # 122880
TAIL = RN - MAIN_ROWS        # 2120
R_TA = TAIL // P             # 16 rows/partition in tail A
TA_ROWS = P * R_TA           # 2048
TB_ROWS = TAIL - TA_ROWS     # 72 (one row on each of 72 partitions)

F32 = mybir.dt.float32

LAST_RESULTS = None  # BassKernelResults of the most recent run (for profiling)


def _emit(ctx: ExitStack, tc, x1, x2, w, fused, alpha):
    nc = tc.nc

    singles = ctx.enter_context(tc.tile_pool(name="singles", bufs=1))
    big = ctx.enter_context(tc.tile_pool(name="big", bufs=3))
    small = ctx.enter_context(tc.tile_pool(name="small", bufs=4))

    # w broadcast to [P, R, D] once: every partition holds R copies of w.
    w_full = singles.tile([P, R, D], F32)
    w_bcast = bass.AP(
        tensor=w.tensor,
        offset=w.offset,
        ap=[[0, P], [0, R]] + list(w.ap),
    )
    nc.sync.dma_start(out=w_full, in_=w_bcast)

    def do_chunk(x1v, x2v, fv, av, parts, rr):
        x1t = big.tile([parts, rr, D], F32, tag="x1t")
        x2t = big.tile([parts, rr, D], F32, tag="x2t")
        dif = big.tile([parts, rr, D], F32, tag="dif")
        prd = big.tile([parts, rr, D], F32, tag="prd")
        scl = big.tile([parts, rr, D], F32, tag="scl")
        fut = big.tile([parts, rr, D], F32, tag="fut")
        sd = small.tile([parts, rr, 1], F32, tag="sd")
        ap2 = small.tile([parts, rr, 2], F32, tag="ap2")

        nc.sync.dma_start(out=x1t, in_=x1v)
        nc.sync.dma_start(out=x2t, in_=x2v)

        nc.vector.tensor_sub(dif, x1t, x2t)
        nc.vector.tensor_mul(prd, dif, w_full[:parts, :rr, :])
        nc.vector.reduce_sum(sd, prd, axis=mybir.AxisListType.X)
        nc.scalar.activation(
            ap2[:, :, 0:1], sd, mybir.ActivationFunctionType.Sigmoid
        )
        nc.scalar.activation(
            ap2[:, :, 1:2], sd, mybir.ActivationFunctionType.Sigmoid, scale=-1.0
        )
        a1b = ap2[:, :, 0:1].broadcast_to([parts, rr, D])
        nc.vector.tensor_mul(scl, dif, a1b)
        nc.vector.tensor_add(fut, scl, x2t)

        nc.sync.dma_start(out=fv, in_=fut)
        nc.sync.dma_start(out=av, in_=ap2)

    x1m = x1[:MAIN_ROWS].rearrange("(c p r) d -> c p r d", p=P, r=R)
    x2m = x2[:MAIN_ROWS].rearrange("(c p r) d -> c p r d", p=P, r=R)
    fm = fused[:MAIN_ROWS].rearrange("(c p r) d -> c p r d", p=P, r=R)
    am = alpha[:MAIN_ROWS].rearrange("(c p r) t -> c p r t", p=P, r=R)
    for c in range(N_CHUNKS):
        do_chunk(x1m[c], x2m[c], fm[c], am[c], P, R)

    # tail A: [128 partitions x 16 rows]
    sa, ea = MAIN_ROWS, MAIN_ROWS + TA_ROWS
    do_chunk(
        x1[sa:ea].rearrange("(p r) d -> p r d", p=P),
        x2[sa:ea].rearrange("(p r) d -> p r d", p=P),
        fused[sa:ea].rearrange("(p r) d -> p r d", p=P),
        alpha[sa:ea].rearrange("(p r) t -> p r t", p=P),
        P,
        R_TA,
    )
    # tail B: [72 partitions x 1 row]
    do_chunk(
        x1[ea:].rearrange("(p r) d -> p r d", p=TB_ROWS),
        x2[ea:].rearrange("(p r) d -> p r d", p=TB_ROWS),
        fused[ea:].rearrange("(p r) d -> p r d", p=TB_ROWS),
        alpha[ea:].rearrange("(p r) t -> p r t", p=TB_ROWS),
        TB_ROWS,
        1,
    )


_cached_nc = None


def _build():
    nc = bacc.Bacc(
        "TRN2", target_bir_lowering=False, debug=False, num_devices=N_CORES
    )
    x1 = nc.dram_tensor("x1", [RN, D], F32, kind="ExternalInput").ap()
    x2 = nc.dram_tensor("x2", [RN, D], F32, kind="ExternalInput").ap()
    w = nc.dram_tensor("w", [D], F32, kind="ExternalInput").ap()
    fused = nc.dram_tensor("fused", [RN, D], F32, kind="ExternalOutput").ap()
    alpha = nc.dram_tensor("alpha", [RN, 2], F32, kind="ExternalOutput").ap()

    with tile.TileContext(nc) as tc, ExitStack() as ctx:
        _emit(ctx, tc, x1, x2, w, fused, alpha)
    nc.compile()
    return nc


def kernel(x1, x2, w, _trace=False):
    global _cached_nc, LAST_RESULTS
    x1 = np.ascontiguousarray(np.asarray(x1, dtype=np.float32))
    x2 = np.ascontiguousarray(np.asarray(x2, dtype=np.float32))
    w = np.ascontiguousarray(np.asarray(w, dtype=np.float32))
    assert x1.shape == (N, D) and x2.shape == (N, D) and w.shape == (D,)

    if _cached_nc is None:
        _cached_nc = _build()
    nc = _cached_nc

    in_maps = [
        {
            "x1": x1[i * RN : (i + 1) * RN],
            "x2": x2[i * RN : (i + 1) * RN],
            "w": w,
        }
        for i in range(N_CORES)
    ]
    res = run_bass_kernel_spmd(
        nc, in_maps, core_ids=list(range(N_CORES)), trace=_trace
    )
    LAST_RESULTS = res
    fused = np.concatenate(
        [res.results[i]["fused"] for i in range(N_CORES)], axis=0
    )
    alpha = np.concatenate(
        [res.results[i]["alpha"] for i in range(N_CORES)], axis=0
    )
    return fused, alpha


# revision 22
# speedup vs baseline: 15.2673x; 15.2673x over previous
"""Trainium2 Bass kernel for nn_AttentionFusion.

reference:
    s1 = x1 @ w; s2 = x2 @ w               # [N]
    a1 = sigmoid(s1 - s2); a2 = 1 - a1     # [N]
    fused = a1[:,None]*x1 + a2[:,None]*x2  # [N, 64]
    alpha = stack([a1, a2], axis=1)        # [N, 2]

Sharding: rows split evenly across 8 NeuronCores (125000 rows/core), w
replicated. Per core the rows are streamed in a flat layout: each SBUF
partition holds R consecutive rows (R*64 contiguous fp32 -> big linear
DMA bursts). Compute per chunk:
    diff  = x1 - x2                       (DVE)
    sdiff = reduce_d(diff * w)            (DVE mul + grouped reduce)
    a1    = sigmoid(sdiff), a2 = sigmoid(-sdiff)   (ACT)
    fused = x2 + a1 * diff                (DVE, a1 broadcast along d)
"""

import numpy as np
from contextlib import ExitStack

import concourse.bass as bass
import concourse.bacc as bacc
import concourse.tile as tile
from concourse import mybir
from concourse.bass_utils import run_bass_kernel_spmd
from concourse.dve_ops import OPS, DveOp, get_dve_sub_opcode, has_src1
from concourse.dve_spec import AluOp, Spec, Src0, Src1, lower, scan
from concourse.dve_uop import DveOpSpec


def _register_wscan():
    """Custom DVE op: out[p, k] = cumsum_k(in0[p, :] * in1[p, :]).

    One 1x fp32 DVE pass produces the weighted prefix sums of diff*w;
    the per-row (64-element group) dot products are then the strided
    differences of the scan, so the separate multiply and grouped-reduce
    passes collapse into this single instruction.
    """
    name = "ATTNFUSE_WSCAN"
    for op in OPS:
        if op.name == name:
            return op

    def _ref(in0, in1, *_args):
        prod = (np.asarray(in0) * np.asarray(in1)).astype(np.float32)
        flat = prod.reshape(prod.shape[0], -1)
        return np.cumsum(flat, axis=-1).astype(np.float32).reshape(prod.shape)

    op = DveOp(
        name,
        Spec(body=scan(AluOp.ADD, Src0 * Src1), reference=_ref),
        subdim=False,
        uops_sha={},
    )
    OPS.append(op)
    # module-level lookup tables are built at import; register there too
    import concourse.dve_ops as _dve_ops_mod

    _dve_ops_mod.CUSTOM_DVE_SPECS[op.name] = op.spec
    _dve_ops_mod._SUB_OPCODE_FOR_NAME[op.name] = (
        _dve_ops_mod._CUSTOM_DVE_ROW_BASE + len(OPS) - 1
    )
    for ver in ("v3", "v4"):
        compiled = DveOpSpec(
            name=op.name,
            opcode=get_dve_sub_opcode(op.name),
            uops=lower(op.spec, ver=ver),
            rd1_en=has_src1(op.spec),
        )
        op.uops_sha[ver] = compiled.sha(ver)
    return op


WSCAN = _register_wscan()

N_CORES = 8
N, D = 1_000_000, 64
RN = N // N_CORES            # 125000 rows per core
P = 128                      # SBUF partitions
R = 64                       # rows per partition per main chunk
CHUNK_ROWS = P * R           # 4096
N_CHUNKS = RN // CHUNK_ROWS  # 30
MAIN_ROWS = N_CHUNKS * CHUNK_ROWS  # 122880
TAIL = RN - MAIN_ROWS        # 2120
R_TA = TAIL // P             # 16 rows/partition in tail A
TA_ROWS = P * R_TA           # 2048
TB_ROWS = TAIL - TA_ROWS     # 72 (one row on each of 72 partitions)

F32 = mybir.dt.float32

COMBINE_MODE = "bcast"  # "stt" (per-row scalar_tensor_tensor) or "bcast"

LAST_RESULTS = None  # BassKernelResults of the most recent run (for profiling)


def _emit(ctx: ExitStack, tc, x1, x2, w, fused, alpha, n_chunks=None, loops=1):
    nc = tc.nc
    do_tails = n_chunks is None
    if n_chunks is None:
        n_chunks = N_CHUNKS

    singles = ctx.enter_context(tc.tile_pool(name="singles", bufs=1))
    big = ctx.enter_context(tc.tile_pool(name="big", bufs=3))
    small = ctx.enter_context(tc.tile_pool(name="small", bufs=4))


    # w broadcast to [P, R, D] once: every partition holds R copies of w.
    w_full = singles.tile([P, R, D], F32)
    w_bcast = bass.AP(
        tensor=w.tensor,
        offset=w.offset,
        ap=[[0, P], [0, R]] + list(w.ap),
    )
    nc.sync.dma_start(out=w_full, in_=w_bcast)

    def do_chunk(x1v, x2v, fv, av, parts, rr):
        x1t = big.tile([parts, rr, D], F32, tag="x1t")
        x2t = big.tile([parts, rr, D], F32, tag="x2t")
        prd = big.tile([parts, rr, D], F32, tag="prd")
        sd = small.tile([parts, rr, 1], F32, tag="sd")
        ap2 = small.tile([parts, rr, 2], F32, tag="ap2")

        nc.sync.dma_start(out=x1t, in_=x1v)
        nc.sync.dma_start(out=x2t, in_=x2v)

        # x1t <- diff = x1 - x2 (in place; elementwise 1:1 is stream-safe)
        nc.vector.tensor_sub(x1t, x1t, x2t)
        # prd = running cumsum of diff*w along the whole free dim; per-row
        # dot products are then strided differences of the scan.
        nc.vector._custom_dve(
            WSCAN,
            out=prd.rearrange("p r d -> p (r d)"),
            in0=x1t.rearrange("p r d -> p (r d)"),
            in1=w_full[:parts, :rr, :].rearrange("p r d -> p (r d)"),
        )
        ce = prd[:, :, D - 1 : D]  # [parts, rr, 1] group-end cumsums
        nc.vector.tensor_copy(sd[:, 0:1, :], ce[:, 0:1, :])
        if rr > 1:
            nc.vector.tensor_sub(sd[:, 1:, :], ce[:, 1:, :], ce[:, :-1, :])
        nc.scalar.activation(
            ap2[:, :, 0:1], sd, mybir.ActivationFunctionType.Sigmoid
        )
        nc.scalar.activation(
            ap2[:, :, 1:2], sd, mybir.ActivationFunctionType.Sigmoid, scale=-1.0
        )
        if COMBINE_MODE == "stt":
            # x1t <- fused = diff*a1 + x2, one scalar_tensor_tensor per
            # row-group: the per-partition scalar slot broadcasts a1 over
            # the 64-elem row.
            for r in range(rr):
                nc.vector.scalar_tensor_tensor(
                    out=x1t[:, r, :],
                    in0=x1t[:, r, :],
                    scalar=ap2[:, r, 0:1],
                    in1=x2t[:, r, :],
                    op0=mybir.AluOpType.mult,
                    op1=mybir.AluOpType.add,
                )
        else:
            # two full-width passes: x1t <- diff * a1 (a1 broadcast along
            # d via step-0 AP), then x1t <- x1t + x2
            a1b = ap2[:, :, 0:1].broadcast_to([parts, rr, D])
            nc.vector.tensor_mul(x1t, x1t, a1b)
            nc.vector.tensor_add(x1t, x1t, x2t)

        nc.sync.dma_start(out=fv, in_=x1t)
        nc.sync.dma_start(out=av, in_=ap2)

    x1m = x1[:MAIN_ROWS].rearrange("(c p r) d -> c p r d", p=P, r=R)
    x2m = x2[:MAIN_ROWS].rearrange("(c p r) d -> c p r d", p=P, r=R)
    fm = fused[:MAIN_ROWS].rearrange("(c p r) d -> c p r d", p=P, r=R)
    am = alpha[:MAIN_ROWS].rearrange("(c p r) t -> c p r t", p=P, r=R)
    for _ in range(loops):
        for c in range(n_chunks):
            do_chunk(x1m[c], x2m[c], fm[c], am[c], P, R)

        if not do_tails:
            continue
        # tail A: [128 partitions x 16 rows]
        sa, ea = MAIN_ROWS, MAIN_ROWS + TA_ROWS
        do_chunk(
            x1[sa:ea].rearrange("(p r) d -> p r d", p=P),
            x2[sa:ea].rearrange("(p r) d -> p r d", p=P),
            fused[sa:ea].rearrange("(p r) d -> p r d", p=P),
            alpha[sa:ea].rearrange("(p r) t -> p r t", p=P),
            P,
            R_TA,
        )
        # tail B: [72 partitions x 1 row]
        do_chunk(
            x1[ea:].rearrange("(p r) d -> p r d", p=TB_ROWS),
            x2[ea:].rearrange("(p r) d -> p r d", p=TB_ROWS),
            fused[ea:].rearrange("(p r) d -> p r d", p=TB_ROWS),
            alpha[ea:].rearrange("(p r) t -> p r t", p=TB_ROWS),
            TB_ROWS,
            1,
        )


_cached_nc = None


def _build(n_chunks=None, loops=1):
    nc = bacc.Bacc(
        "TRN2", target_bir_lowering=False, debug=False, num_devices=N_CORES
    )
    x1 = nc.dram_tensor("x1", [RN, D], F32, kind="ExternalInput").ap()
    x2 = nc.dram_tensor("x2", [RN, D], F32, kind="ExternalInput").ap()
    w = nc.dram_tensor("w", [D], F32, kind="ExternalInput").ap()
    fused = nc.dram_tensor("fused", [RN, D], F32, kind="ExternalOutput").ap()
    alpha = nc.dram_tensor("alpha", [RN, 2], F32, kind="ExternalOutput").ap()

    with tile.TileContext(nc) as tc, ExitStack() as ctx:
        _emit(ctx, tc, x1, x2, w, fused, alpha, n_chunks=n_chunks, loops=loops)
    nc.compile()
    return nc


def kernel(x1, x2, w, _trace=False):
    global _cached_nc, LAST_RESULTS
    x1 = np.ascontiguousarray(np.asarray(x1, dtype=np.float32))
    x2 = np.ascontiguousarray(np.asarray(x2, dtype=np.float32))
    w = np.ascontiguousarray(np.asarray(w, dtype=np.float32))
    assert x1.shape == (N, D) and x2.shape == (N, D) and w.shape == (D,)

    if _cached_nc is None:
        _cached_nc = _build()
    nc = _cached_nc

    in_maps = [
        {
            "x1": x1[i * RN : (i + 1) * RN],
            "x2": x2[i * RN : (i + 1) * RN],
            "w": w,
        }
        for i in range(N_CORES)
    ]
    res = run_bass_kernel_spmd(
        nc, in_maps, core_ids=list(range(N_CORES)), trace=_trace
    )
    LAST_RESULTS = res
    fused = np.concatenate(
        [res.results[i]["fused"] for i in range(N_CORES)], axis=0
    )
    alpha = np.concatenate(
        [res.results[i]["alpha"] for i in range(N_CORES)], axis=0
    )
    return fused, alpha


# revision 24
# speedup vs baseline: 17.1094x; 1.1207x over previous
"""Trainium2 Bass kernel for nn_AttentionFusion.

reference:
    s1 = x1 @ w; s2 = x2 @ w               # [N]
    a1 = sigmoid(s1 - s2); a2 = 1 - a1     # [N]
    fused = a1[:,None]*x1 + a2[:,None]*x2  # [N, 64]
    alpha = stack([a1, a2], axis=1)        # [N, 2]

Sharding: rows split evenly across 8 NeuronCores (125000 rows/core), w
replicated. Per core the rows are streamed in a flat layout: each SBUF
partition holds R consecutive rows (R*64 contiguous fp32 -> big linear
DMA bursts). Compute per chunk:
    diff  = x1 - x2                       (DVE)
    sdiff = reduce_d(diff * w)            (DVE mul + grouped reduce)
    a1    = sigmoid(sdiff), a2 = sigmoid(-sdiff)   (ACT)
    fused = x2 + a1 * diff                (DVE, a1 broadcast along d)
"""

import numpy as np
from contextlib import ExitStack

import concourse.bass as bass
import concourse.bacc as bacc
import concourse.tile as tile
from concourse import mybir
from concourse.bass_utils import run_bass_kernel_spmd
from concourse.dve_ops import OPS, DveOp, get_dve_sub_opcode, has_src1
from concourse.dve_spec import AluOp, Spec, Src0, Src1, lower, scan
from concourse.dve_uop import DveOpSpec


def _register_wscan():
    """Custom DVE op: out[p, k] = cumsum_k(in0[p, :] * in1[p, :]).

    One 1x fp32 DVE pass produces the weighted prefix sums of diff*w;
    the per-row (64-element group) dot products are then the strided
    differences of the scan, so the separate multiply and grouped-reduce
    passes collapse into this single instruction.
    """
    name = "ATTNFUSE_WSCAN"
    for op in OPS:
        if op.name == name:
            return op

    def _ref(in0, in1, *_args):
        prod = (np.asarray(in0) * np.asarray(in1)).astype(np.float32)
        flat = prod.reshape(prod.shape[0], -1)
        return np.cumsum(flat, axis=-1).astype(np.float32).reshape(prod.shape)

    op = DveOp(
        name,
        Spec(body=scan(AluOp.ADD, Src0 * Src1), reference=_ref),
        subdim=False,
        uops_sha={},
    )
    OPS.append(op)
    # module-level lookup tables are built at import; register there too
    import concourse.dve_ops as _dve_ops_mod

    _dve_ops_mod.CUSTOM_DVE_SPECS[op.name] = op.spec
    _dve_ops_mod._SUB_OPCODE_FOR_NAME[op.name] = (
        _dve_ops_mod._CUSTOM_DVE_ROW_BASE + len(OPS) - 1
    )
    for ver in ("v3", "v4"):
        compiled = DveOpSpec(
            name=op.name,
            opcode=get_dve_sub_opcode(op.name),
            uops=lower(op.spec, ver=ver),
            rd1_en=has_src1(op.spec),
        )
        op.uops_sha[ver] = compiled.sha(ver)
    return op


WSCAN = _register_wscan()

N_CORES = 8
N, D = 1_000_000, 64
RN = N // N_CORES            # 125000 rows per core
P = 128                      # SBUF partitions
R = 64                       # rows per partition per main chunk
CHUNK_ROWS = P * R           # 4096
N_CHUNKS = RN // CHUNK_ROWS  # 30
MAIN_ROWS = N_CHUNKS * CHUNK_ROWS  # 122880
TAIL = RN - MAIN_ROWS        # 2120
R_TA = TAIL // P             # 16 rows/partition in tail A
TA_ROWS = P * R_TA           # 2048
TB_ROWS = TAIL - TA_ROWS     # 72 (one row on each of 72 partitions)

F32 = mybir.dt.float32

COMBINE_MODE = "bcast"  # "stt" (per-row scalar_tensor_tensor) or "bcast"

LAST_RESULTS = None  # BassKernelResults of the most recent run (for profiling)


def _emit(ctx: ExitStack, tc, x1, x2, w, fused, alpha, n_chunks=None, loops=1):
    nc = tc.nc
    do_tails = n_chunks is None
    if n_chunks is None:
        n_chunks = N_CHUNKS

    singles = ctx.enter_context(tc.tile_pool(name="singles", bufs=1))
    inputs = ctx.enter_context(tc.tile_pool(name="inputs", bufs=4))
    mid = ctx.enter_context(tc.tile_pool(name="mid", bufs=2))
    small = ctx.enter_context(tc.tile_pool(name="small", bufs=4))


    # w broadcast to [P, R, D] once: every partition holds R copies of w.
    w_full = singles.tile([P, R, D], F32)
    w_bcast = bass.AP(
        tensor=w.tensor,
        offset=w.offset,
        ap=[[0, P], [0, R]] + list(w.ap),
    )
    nc.sync.dma_start(out=w_full, in_=w_bcast)

    def do_chunk(x1v, x2v, fv, av, parts, rr):
        x1t = inputs.tile([parts, rr, D], F32, tag="x1t")
        x2t = inputs.tile([parts, rr, D], F32, tag="x2t")
        prd = mid.tile([parts, rr, D], F32, tag="prd")
        sd = small.tile([parts, rr, 1], F32, tag="sd")
        ap2 = small.tile([parts, rr, 2], F32, tag="ap2")

        nc.sync.dma_start(out=x1t, in_=x1v)
        nc.sync.dma_start(out=x2t, in_=x2v)

        # x1t <- diff = x1 - x2 (in place; elementwise 1:1 is stream-safe)
        nc.vector.tensor_sub(x1t, x1t, x2t)
        # prd = running cumsum of diff*w along the whole free dim; per-row
        # dot products are then strided differences of the scan.
        nc.vector._custom_dve(
            WSCAN,
            out=prd.rearrange("p r d -> p (r d)"),
            in0=x1t.rearrange("p r d -> p (r d)"),
            in1=w_full[:parts, :rr, :].rearrange("p r d -> p (r d)"),
        )
        ce = prd[:, :, D - 1 : D]  # [parts, rr, 1] group-end cumsums
        nc.vector.tensor_copy(sd[:, 0:1, :], ce[:, 0:1, :])
        if rr > 1:
            nc.vector.tensor_sub(sd[:, 1:, :], ce[:, 1:, :], ce[:, :-1, :])
        nc.scalar.activation(
            ap2[:, :, 0:1], sd, mybir.ActivationFunctionType.Sigmoid
        )
        nc.scalar.activation(
            ap2[:, :, 1:2], sd, mybir.ActivationFunctionType.Sigmoid, scale=-1.0
        )
        if COMBINE_MODE == "stt":
            # x1t <- fused = diff*a1 + x2, one scalar_tensor_tensor per
            # row-group: the per-partition scalar slot broadcasts a1 over
            # the 64-elem row.
            for r in range(rr):
                nc.vector.scalar_tensor_tensor(
                    out=x1t[:, r, :],
                    in0=x1t[:, r, :],
                    scalar=ap2[:, r, 0:1],
                    in1=x2t[:, r, :],
                    op0=mybir.AluOpType.mult,
                    op1=mybir.AluOpType.add,
                )
        else:
            # two full-width passes: x1t <- diff * a1 (a1 broadcast along
            # d via step-0 AP), then x1t <- x1t + x2
            a1b = ap2[:, :, 0:1].broadcast_to([parts, rr, D])
            nc.vector.tensor_mul(x1t, x1t, a1b)
            nc.vector.tensor_add(x1t, x1t, x2t)

        nc.sync.dma_start(out=fv, in_=x1t)
        nc.sync.dma_start(out=av, in_=ap2)

    x1m = x1[:MAIN_ROWS].rearrange("(c p r) d -> c p r d", p=P, r=R)
    x2m = x2[:MAIN_ROWS].rearrange("(c p r) d -> c p r d", p=P, r=R)
    fm = fused[:MAIN_ROWS].rearrange("(c p r) d -> c p r d", p=P, r=R)
    am = alpha[:MAIN_ROWS].rearrange("(c p r) t -> c p r t", p=P, r=R)
    for _ in range(loops):
        for c in range(n_chunks):
            do_chunk(x1m[c], x2m[c], fm[c], am[c], P, R)

        if not do_tails:
            continue
        # tail A: [128 partitions x 16 rows]
        sa, ea = MAIN_ROWS, MAIN_ROWS + TA_ROWS
        do_chunk(
            x1[sa:ea].rearrange("(p r) d -> p r d", p=P),
            x2[sa:ea].rearrange("(p r) d -> p r d", p=P),
            fused[sa:ea].rearrange("(p r) d -> p r d", p=P),
            alpha[sa:ea].rearrange("(p r) t -> p r t", p=P),
            P,
            R_TA,
        )
        # tail B: [72 partitions x 1 row]
        do_chunk(
            x1[ea:].rearrange("(p r) d -> p r d", p=TB_ROWS),
            x2[ea:].rearrange("(p r) d -> p r d", p=TB_ROWS),
            fused[ea:].rearrange("(p r) d -> p r d", p=TB_ROWS),
            alpha[ea:].rearrange("(p r) t -> p r t", p=TB_ROWS),
            TB_ROWS,
            1,
        )


_cached_nc = None


def _build(n_chunks=None, loops=1):
    nc = bacc.Bacc(
        "TRN2", target_bir_lowering=False, debug=False, num_devices=N_CORES
    )
    x1 = nc.dram_tensor("x1", [RN, D], F32, kind="ExternalInput").ap()
    x2 = nc.dram_tensor("x2", [RN, D], F32, kind="ExternalInput").ap()
    w = nc.dram_tensor("w", [D], F32, kind="ExternalInput").ap()
    fused = nc.dram_tensor("fused", [RN, D], F32, kind="ExternalOutput").ap()
    alpha = nc.dram_tensor("alpha", [RN, 2], F32, kind="ExternalOutput").ap()

    with tile.TileContext(nc) as tc, ExitStack() as ctx:
        _emit(ctx, tc, x1, x2, w, fused, alpha, n_chunks=n_chunks, loops=loops)
    nc.compile()
    return nc


def kernel(x1, x2, w, _trace=False):
    global _cached_nc, LAST_RESULTS
    x1 = np.ascontiguousarray(np.asarray(x1, dtype=np.float32))
    x2 = np.ascontiguousarray(np.asarray(x2, dtype=np.float32))
    w = np.ascontiguousarray(np.asarray(w, dtype=np.float32))
    assert x1.shape == (N, D) and x2.shape == (N, D) and w.shape == (D,)

    if _cached_nc is None:
        _cached_nc = _build()
    nc = _cached_nc

    in_maps = [
        {
            "x1": x1[i * RN : (i + 1) * RN],
            "x2": x2[i * RN : (i + 1) * RN],
            "w": w,
        }
        for i in range(N_CORES)
    ]
    res = run_bass_kernel_spmd(
        nc, in_maps, core_ids=list(range(N_CORES)), trace=_trace
    )
    LAST_RESULTS = res
    fused = np.concatenate(
        [res.results[i]["fused"] for i in range(N_CORES)], axis=0
    )
    alpha = np.concatenate(
        [res.results[i]["alpha"] for i in range(N_CORES)], axis=0
    )
    return fused, alpha
